# revision 15
# baseline (speedup 1.0000x reference)
"""Trainium2 distributed kernel for nn_Attention (dense transformer attention block).

Strategy (8 NeuronCores, tensor-parallel over heads):
  - Host pre-transposes x_norm -> X^T [C, B*T] (bf16) and slices Wqkv columns
    per core (2 heads/core, deinterleaved RoPE feature order). RoPE sin/cos
    tables precomputed host-side.
  - Each core computes, in bf16 on the TensorEngine:
      1) Q^T/K^T (head-major, D on partitions) + V (natural) for its 2 heads,
         with bias + RoPE fused into the epilogue.
      2) Causal flash attention without max-subtraction (scores ~ N(0,1)):
         S = Q^T.T @ K^T chunks -> exp (with 1/sqrt(D) scale + causal mask)
         -> row-sums via ACT accum -> normalize P -> PE-transpose P blocks
         -> out^T = V.T @ P^T accumulated in PSUM (+V-bias via P-rows-sum-to-1).
      3) Tiny AllToAll (4 MiB bf16) of out^T row-shards -> every core gets its
         (B*T/8)-row shard with all 16 heads = X2^T [C, B*T/8].
      4) Local out-projection X2 @ Wout (+bout via rank-1 matmul) -> fp32 out.
  - Host concatenates the 8 row-shards -> [B, T, C] fp32.
"""

import numpy as np
import ml_dtypes

import concourse.bass as bass
import concourse.mybir as mybir
import concourse.tile as tile
from concourse import bacc
from concourse.bass_utils import run_bass_kernel_spmd
from concourse.masks import make_identity

N_CORES = 8
B, T, C = 4, 2048, 2048
H, D = 16, 128
ROPE_BASE = 10000.0

BF16 = mybir.dt.bfloat16
F32 = mybir.dt.float32
NPBF16 = ml_dtypes.bfloat16


def _stage1(nc, tc, p, qT_sb, kT_sb, v_sb, wq_sb, wk_sb, wv_sb,
            bq_sb, bk_sb, cos_sb, sin_sb, xT):
    """QKV projection + bias + RoPE into resident SBUF."""
    RC, n_rc, KT, HL, t = p["RC"], p["n_rc"], p["KT"], p["HL"], p["t"]
    # ---- Q^T / K^T ----
    with (
        tc.tile_pool(name="xin_a", bufs=4) as xin,
        tc.tile_pool(name="ps_a", bufs=2 * 2 * HL, space="PSUM") as psa,
        tc.tile_pool(name="rope", bufs=4) as ropetmp,
    ):
        for rc in range(n_rc):
            r0 = rc * RC
            t0 = r0 % t
            psq = [psa.tile([128, RC], F32, tag="ps_qk", name=f"psq{rc}_{i}")
                   for i in range(2 * HL)]
            for kt in range(KT):
                xt = xin.tile([128, RC], BF16, tag="xt")
                nc.sync.dma_start(out=xt, in_=xT[kt * 128:(kt + 1) * 128, r0:r0 + RC])
                for hm in range(HL):
                    nc.tensor.matmul(psq[hm], lhsT=wq_sb[:, kt, hm * 128:(hm + 1) * 128],
                                     rhs=xt, start=(kt == 0), stop=(kt == KT - 1))
                    nc.tensor.matmul(psq[HL + hm], lhsT=wk_sb[:, kt, hm * 128:(hm + 1) * 128],
                                     rhs=xt, start=(kt == 0), stop=(kt == KT - 1))
            for which, (res, bias_sb) in enumerate(((qT_sb, bq_sb), (kT_sb, bk_sb))):
                for hm in range(HL):
                    dst = res[:, hm, r0:r0 + RC]
                    ps = psq[which * HL + hm]
                    nc.scalar.activation(out=dst, in_=ps,
                                         func=mybir.ActivationFunctionType.Identity,
                                         bias=bias_sb[:, hm:hm + 1], scale=1.0)
                    # RoPE in place: pairs (j, 64+j), angle t*w_j
                    x0 = res[0:64, hm, r0:r0 + RC]
                    x1 = res[64:128, hm, r0:r0 + RC]
                    rt = ropetmp.tile([128, RC], BF16, tag="rt")
                    nc.vector.tensor_mul(rt[0:64, :], x1, sin_sb[64:128, t0:t0 + RC])
                    nc.vector.tensor_mul(rt[64:128, :], x0, sin_sb[0:64, t0:t0 + RC])
                    nc.vector.tensor_mul(x0, x0, cos_sb[0:64, t0:t0 + RC])
                    nc.vector.tensor_sub(x0, x0, rt[0:64, :])
                    nc.vector.tensor_mul(x1, x1, cos_sb[64:128, t0:t0 + RC])
                    nc.vector.tensor_add(x1, x1, rt[64:128, :])
    # ---- V (natural layout) ----
    with (
        tc.tile_pool(name="xin_b", bufs=4) as xin,
        tc.tile_pool(name="ps_b", bufs=2 * (RC // 128), space="PSUM") as psb,
    ):
        for rc in range(n_rc):
            r0 = rc * RC
            psv = [psb.tile([128, p["HD"]], F32, tag="ps_v", name=f"psv{rc}_{i}")
                   for i in range(RC // 128)]
            for kt in range(KT):
                xt = xin.tile([128, RC], BF16, tag="xt")
                nc.sync.dma_start(out=xt, in_=xT[kt * 128:(kt + 1) * 128, r0:r0 + RC])
                for rs_ in range(RC // 128):
                    nc.tensor.matmul(psv[rs_], lhsT=xt[:, rs_ * 128:(rs_ + 1) * 128],
                                     rhs=wv_sb[:, kt, :], start=(kt == 0), stop=(kt == KT - 1))
            for rs_ in range(RC // 128):
                rt_ = (r0 // 128) + rs_
                nc.scalar.activation(out=v_sb[:, rt_, :], in_=psv[rs_],
                                     func=mybir.ActivationFunctionType.Copy, scale=1.0)


def _stage2(nc, tc, p, qT_sb, kT_sb, v_sb, bv_sb, mask_sb, ident, a2a_in):
    """Causal attention per (batch, local head); writes a2a_in."""
    import os
    SUB = int(os.environ.get("SUBSTAGE", "4"))
    b, t, HL, HD, d, RS, NQT, NSEG = (p["b"], p["t"], p["HL"], p["HD"], p["d"],
                                      p["RS"], p["NQT"], p["NSEG"])
    SCALE = p["SCALE"]
    with (
        tc.tile_pool(name="att", bufs=3) as att,
        tc.tile_pool(name="attsm", bufs=6) as attsm,
        tc.tile_pool(name="spsum", bufs=2, space="PSUM") as spsum,
        tc.tile_pool(name="tpsum", bufs=3, space="PSUM") as tpsum,
        tc.tile_pool(name="opsum", bufs=2, space="PSUM") as opsum,
        tc.tile_pool(name="oTp", bufs=2) as oTpool,
    ):
        for bb in range(b):
            for hm in range(HL):
                qT_h = qT_sb[:, hm, bb * t:(bb + 1) * t]
                kT_h = kT_sb[:, hm, bb * t:(bb + 1) * t]
                oT = oTpool.tile([128, t], BF16, tag="oT")
                for iq in range(NQT):
                    W = (iq + 1) * 128
                    nchunks = (W + 511) // 512
                    rs_cols = attsm.tile([128, 8], F32, tag="rs")
                    p_sb = att.tile([128, t], BF16, tag="p")
                    ncol = 0
                    for jc in range(nchunks):
                        c0 = jc * 512
                        cw = min(512, W - c0)
                        ps = spsum.tile([128, 512], F32, tag="s")
                        nc.tensor.matmul(ps[:, :cw], lhsT=qT_h[:, iq * 128:(iq + 1) * 128],
                                         rhs=kT_h[:, c0:c0 + cw], start=True, stop=True)
                        if SUB < 1:
                            nc.scalar.activation(out=p_sb[:, c0:c0 + cw], in_=ps[:, :cw],
                                                 func=mybir.ActivationFunctionType.Exp,
                                                 scale=SCALE)
                            continue
                        is_last = (c0 + cw == W)
                        if not is_last:
                            nc.scalar.activation(out=p_sb[:, c0:c0 + cw], in_=ps[:, :cw],
                                                 func=mybir.ActivationFunctionType.Exp,
                                                 scale=SCALE,
                                                 accum_out=rs_cols[:, ncol:ncol + 1])
                            ncol += 1
                        else:
                            woff = cw - 128
                            if woff > 0:
                                nc.scalar.activation(out=p_sb[:, c0:c0 + woff], in_=ps[:, :woff],
                                                     func=mybir.ActivationFunctionType.Exp,
                                                     scale=SCALE,
                                                     accum_out=rs_cols[:, ncol:ncol + 1])
                                ncol += 1
                            tmp_d = attsm.tile([128, 128], BF16, tag="tmpd")
                            nc.scalar.activation(out=tmp_d, in_=ps[:, woff:cw],
                                                 func=mybir.ActivationFunctionType.Exp,
                                                 scale=SCALE)
                            import os as _os
                            if _os.environ.get("TTR", "0") == "1":
                                nc.vector.tensor_tensor_reduce(
                                    out=p_sb[:, W - 128:W], in0=tmp_d, in1=mask_sb,
                                    scale=1.0, scalar=0.0,
                                    op0=mybir.AluOpType.mult, op1=mybir.AluOpType.add,
                                    accum_out=rs_cols[:, ncol:ncol + 1])
                            else:
                                nc.vector.tensor_mul(p_sb[:, W - 128:W], tmp_d, mask_sb)
                                nc.vector.reduce_sum(out=rs_cols[:, ncol:ncol + 1],
                                                     in_=p_sb[:, W - 128:W],
                                                     axis=mybir.AxisListType.X)
                            ncol += 1
                    if SUB < 2:
                        nc.scalar.activation(out=oT[:, iq * 128:(iq + 1) * 128],
                                             in_=p_sb[:, W - 128:W],
                                             func=mybir.ActivationFunctionType.Copy,
                                             scale=1.0)
                        continue
                    rtot = attsm.tile([128, 1], F32, tag="rtot")
                    nc.vector.reduce_sum(out=rtot, in_=rs_cols[:, :ncol],
                                         axis=mybir.AxisListType.X)
                    recip = attsm.tile([128, 1], F32, tag="recip")
                    nc.vector.reciprocal(out=recip, in_=rtot)
                    nc.vector.tensor_scalar_mul(p_sb[:, :W], p_sb[:, :W], recip)
                    if SUB < 3:
                        nc.scalar.activation(out=oT[:, iq * 128:(iq + 1) * 128],
                                             in_=p_sb[:, W - 128:W],
                                             func=mybir.ActivationFunctionType.Copy,
                                             scale=1.0)
                        continue
                    # transpose P blocks + PV accumulate
                    po = opsum.tile([128, 128], F32, tag="po")
                    for jt in range(iq + 1):
                        pt_ps = tpsum.tile([128, 128], BF16, tag="pt")
                        nc.tensor.transpose(pt_ps, p_sb[:, jt * 128:(jt + 1) * 128], ident)
                        pt_sb = attsm.tile([128, 128], BF16, tag="ptsb")
                        if jt % 2 == 0:
                            nc.scalar.activation(out=pt_sb, in_=pt_ps,
                                                 func=mybir.ActivationFunctionType.Copy,
                                                 scale=1.0)
                        else:
                            nc.vector.tensor_copy(pt_sb, pt_ps)
                        if SUB >= 4:
                            nc.tensor.matmul(po, lhsT=v_sb[:, (bb * t) // 128 + jt, hm * d:(hm + 1) * d],
                                             rhs=pt_sb, start=(jt == 0), stop=(jt == iq))
                    if SUB >= 4:
                        nc.scalar.activation(out=oT[:, iq * 128:(iq + 1) * 128], in_=po,
                                             func=mybir.ActivationFunctionType.Identity,
                                             bias=bv_sb[:, hm:hm + 1], scale=1.0)
                    else:
                        nc.scalar.activation(out=oT[:, iq * 128:(iq + 1) * 128],
                                             in_=pt_sb,
                                             func=mybir.ActivationFunctionType.Copy,
                                             scale=1.0)
                for tt in range(NSEG):
                    slot = (bb * t) // RS + tt
                    nc.sync.dma_start(
                        out=a2a_in[slot * HD + hm * d: slot * HD + (hm + 1) * d, :],
                        in_=oT[:, tt * RS:(tt + 1) * RS])


def _stage3(nc, tc, p, a2a_out, wo, bo_sb, ones1, out):
    """A2A result -> out projection -> fp32 output shard."""
    c, KT, RS = p["c"], p["KT"], p["RS"]
    with (
        tc.tile_pool(name="x2", bufs=1) as x2pool,
        tc.tile_pool(name="wop", bufs=KT + 2) as wop,
        tc.tile_pool(name="p3", bufs=4, space="PSUM") as p3pool,
        tc.tile_pool(name="o3", bufs=4) as o3pool,
    ):
        x2T = x2pool.tile([128, KT, RS], BF16, tag="x2T")
        nc.sync.dma_start(out=x2T, in_=a2a_out[:, :].rearrange("(kt p) r -> p kt r", p=128))
        for nn_ in range(c // 512):
            wo_tiles = []
            for kt in range(KT):
                wt = wop.tile([128, 512], BF16, tag="wo", name=f"wo{nn_}_{kt}")
                nc.sync.dma_start(out=wt, in_=wo[kt * 128:(kt + 1) * 128,
                                                nn_ * 512:(nn_ + 1) * 512])
                wo_tiles.append(wt)
            for m in range(RS // 128):
                ps3 = p3pool.tile([128, 512], F32, tag="ps3")
                for kt in range(KT):
                    nc.tensor.matmul(ps3, lhsT=x2T[:, kt, m * 128:(m + 1) * 128],
                                     rhs=wo_tiles[kt], start=(kt == 0), stop=False)
                nc.tensor.matmul(ps3, lhsT=ones1[0:1, :],
                                 rhs=bo_sb[0:1, nn_ * 512:(nn_ + 1) * 512],
                                 start=False, stop=True)
                o3 = o3pool.tile([128, 512], F32, tag="o3")
                nc.scalar.activation(out=o3, in_=ps3,
                                     func=mybir.ActivationFunctionType.Copy, scale=1.0)
                nc.sync.dma_start(out=out[m * 128:(m + 1) * 128,
                                          nn_ * 512:(nn_ + 1) * 512], in_=o3)


def build_nc(b=B, t=T, c=C, h=H, d=D, n_cores=N_CORES, stages=3):
    HL = h // n_cores          # heads per core
    R = b * t                  # total rows
    RS = R // n_cores          # rows per core after A2A
    RC = 512                   # row-chunk for stage 1
    p = dict(b=b, t=t, c=c, h=h, d=d, HL=HL, R=R, RS=RS, RC=RC,
             n_rc=R // RC, KT=c // 128, NQT=t // 128, HD=HL * d,
             NSEG=t // RS, SCALE=1.0 / float(np.sqrt(d)))

    nc = bacc.Bacc(None, target_bir_lowering=False, debug=False,
                   num_devices=n_cores)

    xT = nc.declare_dram_parameter("xT", [c, R], BF16, isOutput=False)
    wq = nc.declare_dram_parameter("wq", [c, p["HD"]], BF16, isOutput=False)
    wk = nc.declare_dram_parameter("wk", [c, p["HD"]], BF16, isOutput=False)
    wv = nc.declare_dram_parameter("wv", [c, p["HD"]], BF16, isOutput=False)
    bq = nc.declare_dram_parameter("bq", [128, HL], F32, isOutput=False)
    bk = nc.declare_dram_parameter("bk", [128, HL], F32, isOutput=False)
    bv = nc.declare_dram_parameter("bv", [128, HL], F32, isOutput=False)
    wo = nc.declare_dram_parameter("wo", [c, c], BF16, isOutput=False)
    bo = nc.declare_dram_parameter("bo", [1, c], BF16, isOutput=False)
    cosT = nc.declare_dram_parameter("cosT", [128, t], BF16, isOutput=False)
    sinT = nc.declare_dram_parameter("sinT", [128, t], BF16, isOutput=False)
    maskc = nc.declare_dram_parameter("maskc", [128, 128], BF16, isOutput=False)
    out = nc.declare_dram_parameter("out", [RS, c], F32, isOutput=True)

    with tile.TileContext(nc) as tc:
        with (
            tc.tile_pool(name="consts", bufs=1) as consts,
            tc.tile_pool(name="qkvres", bufs=1) as qkvres,
            tc.tile_pool(name="dram", bufs=1, space="DRAM") as dram,
        ):
            # ---- constants into SBUF ----
            wq_sb = consts.tile([128, p["KT"], p["HD"]], BF16, tag="wq")
            wk_sb = consts.tile([128, p["KT"], p["HD"]], BF16, tag="wk")
            wv_sb = consts.tile([128, p["KT"], p["HD"]], BF16, tag="wv")
            nc.sync.dma_start(out=wq_sb, in_=wq[:, :].rearrange("(kt p) n -> p kt n", p=128))
            nc.sync.dma_start(out=wk_sb, in_=wk[:, :].rearrange("(kt p) n -> p kt n", p=128))
            nc.sync.dma_start(out=wv_sb, in_=wv[:, :].rearrange("(kt p) n -> p kt n", p=128))
            bq_sb = consts.tile([128, HL], F32, tag="bq")
            bk_sb = consts.tile([128, HL], F32, tag="bk")
            bv_sb = consts.tile([128, HL], F32, tag="bv")
            nc.sync.dma_start(out=bq_sb, in_=bq[:, :])
            nc.sync.dma_start(out=bk_sb, in_=bk[:, :])
            nc.sync.dma_start(out=bv_sb, in_=bv[:, :])
            cos_sb = consts.tile([128, t], BF16, tag="cos")
            sin_sb = consts.tile([128, t], BF16, tag="sin")
            nc.sync.dma_start(out=cos_sb, in_=cosT[:, :])
            nc.sync.dma_start(out=sin_sb, in_=sinT[:, :])
            mask_sb = consts.tile([128, 128], BF16, tag="mask")
            nc.sync.dma_start(out=mask_sb, in_=maskc[:, :])
            ident = consts.tile([128, 128], BF16, tag="ident")
            make_identity(nc, ident)
            bo_sb = consts.tile([1, c], BF16, tag="bo")
            nc.sync.dma_start(out=bo_sb, in_=bo[:, :])
            ones1 = consts.tile([1, 128], BF16, tag="ones1")
            nc.vector.memset(ones1, 1.0)

            # ---- resident QKV (bf16) ----
            qT_sb = qkvres.tile([128, HL, p["R"]], BF16, tag="qT")   # [d, h, row]
            kT_sb = qkvres.tile([128, HL, p["R"]], BF16, tag="kT")
            v_sb = qkvres.tile([128, p["R"] // 128, p["HD"]], BF16, tag="v")

            _stage1(nc, tc, p, qT_sb, kT_sb, v_sb, wq_sb, wk_sb, wv_sb,
                    bq_sb, bk_sb, cos_sb, sin_sb, xT)

            a2a_in = dram.tile([n_cores * p["HD"], RS], BF16, tag="a2a_in")
            a2a_out = dram.tile([n_cores * p["HD"], RS], BF16, tag="a2a_out")

            if stages >= 2:
                _stage2(nc, tc, p, qT_sb, kT_sb, v_sb, bv_sb, mask_sb, ident, a2a_in)
            else:
                for sl in range(n_cores):
                    nc.sync.dma_start(out=a2a_in[sl * p["HD"]:sl * p["HD"] + 128, :],
                                      in_=qT_sb[:, 0, sl * RS:(sl + 1) * RS])

            if stages >= 3:
                nc.gpsimd.collective_compute(
                    "AllToAll", mybir.AluOpType.bypass,
                    replica_groups=[list(range(n_cores))],
                    ins=[a2a_in[:, :].opt()],
                    outs=[a2a_out[:, :].opt()],
                )
                _stage3(nc, tc, p, a2a_out, wo, bo_sb, ones1, out)
            else:
                # debug: dump a2a_in (first HD cols) to out
                with tc.tile_pool(name="dbg", bufs=2) as dbg:
                    for m in range(RS // 128):
                        dt_ = dbg.tile([128, c], F32, tag="dbgt")
                        nc.vector.memset(dt_, 0.0)
                        nc.gpsimd.dma_start(out=dt_[:, 0:RS],
                                            in_=a2a_in[m * 128:(m + 1) * 128, :])
                        nc.sync.dma_start(out=out[m * 128:(m + 1) * 128, :], in_=dt_)

    nc.compile()
    return nc


def _host_prep(x_norm, Wqkv, bqkv, Wout, bout, b, t, c, h, d, n_cores):
    """Build per-core input maps (numpy, bf16)."""
    HL = h // n_cores
    R = b * t
    perm = np.concatenate([np.arange(0, d, 2), np.arange(1, d, 2)])  # deinterleave

    XT = np.ascontiguousarray(x_norm.reshape(R, c).T.astype(NPBF16))
    inv_freq = 1.0 / (ROPE_BASE ** (np.arange(0, d, 2, dtype=np.float64) / d))
    ang = np.arange(t, dtype=np.float64)[None, :] * inv_freq[:, None]  # [d/2, t]
    cosT = np.concatenate([np.cos(ang), np.cos(ang)], axis=0).astype(NPBF16)
    sinT = np.concatenate([np.sin(ang), np.sin(ang)], axis=0).astype(NPBF16)
    maskc = np.tril(np.ones((128, 128), dtype=np.float32)).astype(NPBF16)
    wo_b = np.ascontiguousarray(Wout.astype(NPBF16))
    bo_b = bout.reshape(1, c).astype(NPBF16)

    in_maps = []
    for i in range(n_cores):
        cols_q = np.concatenate([i * HL * d + hh * d + perm for hh in range(HL)])
        cols_k = cols_q + h * d
        cols_v = np.concatenate([2 * h * d + i * HL * d + hh * d + np.arange(d)
                                 for hh in range(HL)])
        wq_i = np.ascontiguousarray(Wqkv[:, cols_q].astype(NPBF16))
        wk_i = np.ascontiguousarray(Wqkv[:, cols_k].astype(NPBF16))
        wv_i = np.ascontiguousarray(Wqkv[:, cols_v].astype(NPBF16))
        bq_i = np.stack([bqkv[i * HL * d + hh * d + perm] for hh in range(HL)],
                        axis=1).astype(np.float32)
        bk_i = np.stack([bqkv[h * d + i * HL * d + hh * d + perm] for hh in range(HL)],
                        axis=1).astype(np.float32)
        bv_i = np.stack([bqkv[2 * h * d + i * HL * d + hh * d + np.arange(d)]
                         for hh in range(HL)], axis=1).astype(np.float32)
        in_maps.append({
            "xT": XT, "wq": wq_i, "wk": wk_i, "wv": wv_i,
            "bq": np.ascontiguousarray(bq_i), "bk": np.ascontiguousarray(bk_i),
            "bv": np.ascontiguousarray(bv_i),
            "wo": wo_b, "bo": bo_b, "cosT": cosT, "sinT": sinT, "maskc": maskc,
        })
    return in_maps


_NC_CACHE = {}


def kernel(x_norm, Wqkv, bqkv, Wout, bout):
    b, t, c = x_norm.shape
    h = 16
    d = c // h
    key = (b, t, c)
    if key not in _NC_CACHE:
        _NC_CACHE[key] = build_nc(b, t, c, h, d, N_CORES)
    nc = _NC_CACHE[key]
    in_maps = _host_prep(np.asarray(x_norm, dtype=np.float32),
                         np.asarray(Wqkv, dtype=np.float32),
                         np.asarray(bqkv, dtype=np.float32),
                         np.asarray(Wout, dtype=np.float32),
                         np.asarray(bout, dtype=np.float32),
                         b, t, c, h, d, N_CORES)
    res = run_bass_kernel_spmd(nc, in_maps, core_ids=list(range(N_CORES)))
    parts = [np.asarray(res.results[i]["out"], dtype=np.float32) for i in range(N_CORES)]
    full = np.concatenate(parts, axis=0)  # [R, C]
    return full.reshape(b, t, c)


# revision 17
# speedup vs baseline: 1.1936x; 1.1936x over previous
"""Trainium2 distributed kernel for nn_Attention (dense transformer attention block).

Strategy (8 NeuronCores, tensor-parallel over heads):
  - Host pre-transposes x_norm -> X^T [C, B*T] (bf16) and slices Wqkv columns
    per core (2 heads/core, deinterleaved RoPE feature order). RoPE sin/cos
    tables precomputed host-side.
  - Each core computes, in bf16 on the TensorEngine:
      1) Q^T/K^T (head-major, D on partitions) + V (natural) for its 2 heads,
         with bias + RoPE fused into the epilogue.
      2) Causal attention, "S^T" flash form without max-subtraction
         (scores ~ N(0,1)): for each K-tile jt and Tq-chunk c:
         S^T[tk, tq] = kT[jt].T @ qT-chunk -> exp (ACT, with 1/sqrt(D) scale,
         triangular mask on the diagonal block) -> P^T tile (SBUF bf16).
         Then two accumulating matmuls per tile: out^T += V[jt].T @ P^T and
         rowsums += ones.T @ P^T (broadcast row-sums on all 128 partitions).
         Normalize with a reciprocal multiply, add V-bias (P rows sum to 1).
      3) Per-batch AllToAll (1 MiB bf16) of out^T row-slices, overlapped with
         the next batch's attention.
      4) Per-batch local out-projection X2 @ Wout (+bout via rank-1 matmul).
  - Host reassembles the per-(core, batch) row pieces -> [B, T, C] fp32.
"""

import numpy as np
import ml_dtypes

import concourse.bass as bass
import concourse.mybir as mybir
import concourse.tile as tile
from concourse import bacc
from concourse.bass_utils import run_bass_kernel_spmd
from concourse.masks import make_identity

N_CORES = 8
B, T, C = 4, 2048, 2048
H, D = 16, 128
ROPE_BASE = 10000.0

BF16 = mybir.dt.bfloat16
F32 = mybir.dt.float32
NPBF16 = ml_dtypes.bfloat16


def _stage1(nc, tc, p, qT_sb, kT_sb, v_sb, wq_sb, wk_sb, wv_sb,
            bq_sb, bk_sb, cos_sb, sin_sb, xT):
    """QKV projection + bias + RoPE into resident SBUF."""
    RC, n_rc, KT, HL, t = p["RC"], p["n_rc"], p["KT"], p["HL"], p["t"]
    dma_engs = [nc.sync, nc.scalar]
    # ---- Q^T / K^T ----
    with (
        tc.tile_pool(name="xin_a", bufs=6) as xin,
        tc.tile_pool(name="ps_a", bufs=2 * 2 * HL, space="PSUM") as psa,
        tc.tile_pool(name="rope", bufs=4) as ropetmp,
    ):
        for rc in range(n_rc):
            r0 = rc * RC
            t0 = r0 % t
            psq = [psa.tile([128, RC], F32, tag="ps_qk", name=f"psq{rc}_{i}")
                   for i in range(2 * HL)]
            for kt in range(KT):
                xt = xin.tile([128, RC], BF16, tag="xt")
                dma_engs[kt % 2].dma_start(out=xt, in_=xT[kt * 128:(kt + 1) * 128, r0:r0 + RC])
                for hm in range(HL):
                    nc.tensor.matmul(psq[hm], lhsT=wq_sb[:, kt, hm * 128:(hm + 1) * 128],
                                     rhs=xt, start=(kt == 0), stop=(kt == KT - 1))
                    nc.tensor.matmul(psq[HL + hm], lhsT=wk_sb[:, kt, hm * 128:(hm + 1) * 128],
                                     rhs=xt, start=(kt == 0), stop=(kt == KT - 1))
            for which, (res, bias_sb) in enumerate(((qT_sb, bq_sb), (kT_sb, bk_sb))):
                for hm in range(HL):
                    dst = res[:, hm, r0:r0 + RC]
                    ps = psq[which * HL + hm]
                    nc.scalar.activation(out=dst, in_=ps,
                                         func=mybir.ActivationFunctionType.Identity,
                                         bias=bias_sb[:, hm:hm + 1], scale=1.0)
                    # RoPE in place: pairs (j, 64+j), angle t*w_j
                    x0 = res[0:64, hm, r0:r0 + RC]
                    x1 = res[64:128, hm, r0:r0 + RC]
                    rt = ropetmp.tile([128, RC], BF16, tag="rt")
                    nc.vector.tensor_mul(rt[0:64, :], x1, sin_sb[64:128, t0:t0 + RC])
                    nc.vector.tensor_mul(rt[64:128, :], x0, sin_sb[0:64, t0:t0 + RC])
                    nc.vector.tensor_mul(x0, x0, cos_sb[0:64, t0:t0 + RC])
                    nc.vector.tensor_sub(x0, x0, rt[0:64, :])
                    nc.vector.tensor_mul(x1, x1, cos_sb[64:128, t0:t0 + RC])
                    nc.vector.tensor_add(x1, x1, rt[64:128, :])
    # ---- V (natural layout) ----
    with (
        tc.tile_pool(name="xin_b", bufs=6) as xin,
        tc.tile_pool(name="ps_b", bufs=2 * (RC // 128), space="PSUM") as psb,
    ):
        for rc in range(n_rc):
            r0 = rc * RC
            psv = [psb.tile([128, p["HD"]], F32, tag="ps_v", name=f"psv{rc}_{i}")
                   for i in range(RC // 128)]
            for kt in range(KT):
                xt = xin.tile([128, RC], BF16, tag="xt")
                dma_engs[kt % 2].dma_start(out=xt, in_=xT[kt * 128:(kt + 1) * 128, r0:r0 + RC])
                for rs_ in range(RC // 128):
                    nc.tensor.matmul(psv[rs_], lhsT=xt[:, rs_ * 128:(rs_ + 1) * 128],
                                     rhs=wv_sb[:, kt, :], start=(kt == 0), stop=(kt == KT - 1))
            for rs_ in range(RC // 128):
                rt_ = (r0 // 128) + rs_
                nc.scalar.activation(out=v_sb[:, rt_, :], in_=psv[rs_],
                                     func=mybir.ActivationFunctionType.Copy, scale=1.0)


def _attn_batch(nc, p, pools, bb, qT_sb, kT_sb, v_sb, bv_sb, maskU_sb, ones_sb,
                a2a_in_b):
    """S^T-form causal attention for one batch (all local heads) -> a2a_in_b."""
    t, HL, HD, d = p["t"], p["HL"], p["HD"], p["d"]
    SCALE = p["SCALE"]
    NCH = t // 512                      # Tq chunks
    seg = t // N_CORES                  # rows per a2a slot
    att, attsm, spsum, opsum, rspsum, oTpool = pools
    for hm in range(HL):
        qT_h = qT_sb[:, hm, bb * t:(bb + 1) * t]
        kT_h = kT_sb[:, hm, bb * t:(bb + 1) * t]
        oT = oTpool.tile([128, t], BF16, tag="oT", name=f"oT{bb}_{hm}")
        for c in range(NCH):
            tq0 = c * 512
            jt_max = 4 * (c + 1)
            psum_o = opsum.tile([128, 512], F32, tag="po", name=f"po{bb}_{hm}_{c}")
            psum_rs = rspsum.tile([128, 512], F32, tag="prs", name=f"prs{bb}_{hm}_{c}")
            for jt in range(jt_max):
                off = max(0, jt * 128 - tq0)
                w = 512 - off
                ps_st = spsum.tile([128, 512], F32, tag="s", name=f"st{bb}_{hm}_{c}_{jt}")
                nc.tensor.matmul(ps_st[:, off:512],
                                 lhsT=kT_h[:, jt * 128:(jt + 1) * 128],
                                 rhs=qT_h[:, tq0 + off:tq0 + 512],
                                 start=True, stop=True)
                pT = att.tile([128, 512], BF16, tag="pT", name=f"pT{bb}_{hm}_{c}_{jt}")
                if jt < 4 * c:
                    nc.scalar.activation(out=pT[:, :], in_=ps_st[:, :],
                                         func=mybir.ActivationFunctionType.Exp,
                                         scale=SCALE)
                else:
                    # diagonal block: cols [off, off+128) need the triu mask
                    tmp_d = attsm.tile([128, 128], BF16, tag="tmpd")
                    nc.scalar.activation(out=tmp_d, in_=ps_st[:, off:off + 128],
                                         func=mybir.ActivationFunctionType.Exp,
                                         scale=SCALE)
                    nc.vector.tensor_mul(pT[:, off:off + 128], tmp_d, maskU_sb)
                    if off + 128 < 512:
                        nc.scalar.activation(out=pT[:, off + 128:512],
                                             in_=ps_st[:, off + 128:512],
                                             func=mybir.ActivationFunctionType.Exp,
                                             scale=SCALE)
                nc.tensor.matmul(psum_o[:, off:512],
                                 lhsT=v_sb[:, (bb * t) // 128 + jt, hm * d:(hm + 1) * d],
                                 rhs=pT[:, off:512],
                                 start=(jt == 0), stop=(jt == jt_max - 1))
                nc.tensor.matmul(psum_rs[:, off:512], lhsT=ones_sb,
                                 rhs=pT[:, off:512],
                                 start=(jt == 0), stop=(jt == jt_max - 1))
            rcp = attsm.tile([128, 512], F32, tag="rcp")
            nc.vector.reciprocal(out=rcp, in_=psum_rs)
            nc.vector.tensor_mul(oT[:, tq0:tq0 + 512], psum_o, rcp)
            nc.vector.tensor_scalar_add(oT[:, tq0:tq0 + 512], oT[:, tq0:tq0 + 512],
                                        bv_sb[:, hm:hm + 1])
        for sl in range(N_CORES):
            nc.gpsimd.dma_start(
                out=a2a_in_b[sl * HD + hm * d: sl * HD + (hm + 1) * d, :],
                in_=oT[:, sl * seg:(sl + 1) * seg])


def _outproj_batch(nc, p, pools, bb, a2a_out_b, wo, bo_sb, ones1, out):
    """Out-projection for this core's row-piece of batch bb."""
    c, KT = p["c"], p["KT"]
    seg = p["t"] // N_CORES             # rows in this piece
    x2pool, wop, p3pool, o3pool = pools
    x2t = x2pool.tile([128, KT, seg], BF16, tag="x2t", name=f"x2t{bb}")
    nc.sync.dma_start(out=x2t, in_=a2a_out_b[:, :].rearrange("(kt p) r -> p kt r", p=128))
    for nn_ in range(c // 512):
        wo_tiles = []
        for kt in range(KT):
            wt = wop.tile([128, 512], BF16, tag="wo", name=f"wo{bb}_{nn_}_{kt}")
            eng = nc.scalar if kt % 2 else nc.sync
            eng.dma_start(out=wt, in_=wo[kt * 128:(kt + 1) * 128,
                                         nn_ * 512:(nn_ + 1) * 512])
            wo_tiles.append(wt)
        MT = min(128, seg)
        for m in range(seg // MT):
            ps3 = p3pool.tile([MT, 512], F32, tag="ps3", name=f"ps3{bb}_{nn_}_{m}")
            for kt in range(KT):
                nc.tensor.matmul(ps3, lhsT=x2t[:, kt, m * MT:(m + 1) * MT],
                                 rhs=wo_tiles[kt], start=(kt == 0), stop=False)
            nc.tensor.matmul(ps3[0:MT, :], lhsT=ones1[0:1, 0:MT],
                             rhs=bo_sb[0:1, nn_ * 512:(nn_ + 1) * 512],
                             start=False, stop=True)
            o3 = o3pool.tile([MT, 512], F32, tag="o3", name=f"o3{bb}_{nn_}_{m}")
            nc.scalar.activation(out=o3, in_=ps3,
                                 func=mybir.ActivationFunctionType.Copy, scale=1.0)
            nc.sync.dma_start(out=out[bb * seg + m * MT:bb * seg + (m + 1) * MT,
                                      nn_ * 512:(nn_ + 1) * 512], in_=o3)


def build_nc(b=B, t=T, c=C, h=H, d=D, n_cores=N_CORES, stages=3):
    HL = h // n_cores          # heads per core
    R = b * t                  # total rows
    RS = R // n_cores          # rows per core overall
    RC = 512                   # row-chunk for stage 1
    p = dict(b=b, t=t, c=c, h=h, d=d, HL=HL, R=R, RS=RS, RC=RC,
             n_rc=R // RC, KT=c // 128, NQT=t // 128, HD=HL * d,
             SCALE=1.0 / float(np.sqrt(d)))
    seg = t // n_cores

    nc = bacc.Bacc(None, target_bir_lowering=False, debug=False,
                   num_devices=n_cores)

    xT = nc.declare_dram_parameter("xT", [c, R], BF16, isOutput=False)
    wq = nc.declare_dram_parameter("wq", [c, p["HD"]], BF16, isOutput=False)
    wk = nc.declare_dram_parameter("wk", [c, p["HD"]], BF16, isOutput=False)
    wv = nc.declare_dram_parameter("wv", [c, p["HD"]], BF16, isOutput=False)
    bq = nc.declare_dram_parameter("bq", [128, HL], F32, isOutput=False)
    bk = nc.declare_dram_parameter("bk", [128, HL], F32, isOutput=False)
    bv = nc.declare_dram_parameter("bv", [128, HL], F32, isOutput=False)
    wo = nc.declare_dram_parameter("wo", [c, c], BF16, isOutput=False)
    bo = nc.declare_dram_parameter("bo", [1, c], BF16, isOutput=False)
    cosT = nc.declare_dram_parameter("cosT", [128, t], BF16, isOutput=False)
    sinT = nc.declare_dram_parameter("sinT", [128, t], BF16, isOutput=False)
    maskc = nc.declare_dram_parameter("maskc", [128, 128], BF16, isOutput=False)
    out = nc.declare_dram_parameter("out", [RS, c], F32, isOutput=True)

    with tile.TileContext(nc) as tc:
        with (
            tc.tile_pool(name="consts", bufs=1) as consts,
            tc.tile_pool(name="qkvres", bufs=1) as qkvres,
            tc.tile_pool(name="dram", bufs=1, space="DRAM") as dram,
        ):
            # ---- constants into SBUF ----
            wq_sb = consts.tile([128, p["KT"], p["HD"]], BF16, tag="wq")
            wk_sb = consts.tile([128, p["KT"], p["HD"]], BF16, tag="wk")
            wv_sb = consts.tile([128, p["KT"], p["HD"]], BF16, tag="wv")
            nc.sync.dma_start(out=wq_sb, in_=wq[:, :].rearrange("(kt p) n -> p kt n", p=128))
            nc.sync.dma_start(out=wk_sb, in_=wk[:, :].rearrange("(kt p) n -> p kt n", p=128))
            nc.sync.dma_start(out=wv_sb, in_=wv[:, :].rearrange("(kt p) n -> p kt n", p=128))
            bq_sb = consts.tile([128, HL], F32, tag="bq")
            bk_sb = consts.tile([128, HL], F32, tag="bk")
            bv_sb = consts.tile([128, HL], F32, tag="bv")
            nc.sync.dma_start(out=bq_sb, in_=bq[:, :])
            nc.sync.dma_start(out=bk_sb, in_=bk[:, :])
            nc.sync.dma_start(out=bv_sb, in_=bv[:, :])
            cos_sb = consts.tile([128, t], BF16, tag="cos")
            sin_sb = consts.tile([128, t], BF16, tag="sin")
            nc.sync.dma_start(out=cos_sb, in_=cosT[:, :])
            nc.sync.dma_start(out=sin_sb, in_=sinT[:, :])
            maskU_sb = consts.tile([128, 128], BF16, tag="mask")
            nc.sync.dma_start(out=maskU_sb, in_=maskc[:, :])
            bo_sb = consts.tile([1, c], BF16, tag="bo")
            nc.sync.dma_start(out=bo_sb, in_=bo[:, :])
            ones1 = consts.tile([1, 128], BF16, tag="ones1")
            nc.vector.memset(ones1, 1.0)
            ones_sb = consts.tile([128, 128], BF16, tag="ones128")
            nc.vector.memset(ones_sb, 1.0)

            # ---- resident QKV (bf16) ----
            qT_sb = qkvres.tile([128, HL, p["R"]], BF16, tag="qT")   # [d, h, row]
            kT_sb = qkvres.tile([128, HL, p["R"]], BF16, tag="kT")
            v_sb = qkvres.tile([128, p["R"] // 128, p["HD"]], BF16, tag="v")

            _stage1(nc, tc, p, qT_sb, kT_sb, v_sb, wq_sb, wk_sb, wv_sb,
                    bq_sb, bk_sb, cos_sb, sin_sb, xT)

            a2a_ins = []
            a2a_outs = []
            for bb in range(b):
                a2a_ins.append(dram.tile([n_cores * p["HD"], seg], BF16,
                                         tag=f"a2a_in{bb}", name=f"a2a_in{bb}"))
                a2a_outs.append(dram.tile([n_cores * p["HD"], seg], BF16,
                                          tag=f"a2a_out{bb}", name=f"a2a_out{bb}"))

            with (
                tc.tile_pool(name="att", bufs=3) as att,
                tc.tile_pool(name="attsm", bufs=4) as attsm,
                tc.tile_pool(name="spsum", bufs=2, space="PSUM") as spsum,
                tc.tile_pool(name="opsum", bufs=2, space="PSUM") as opsum,
                tc.tile_pool(name="rspsum", bufs=2, space="PSUM") as rspsum,
                tc.tile_pool(name="oTp", bufs=2) as oTpool,
                tc.tile_pool(name="x2", bufs=2) as x2pool,
                tc.tile_pool(name="wop", bufs=p["KT"] + 4) as wop,
                tc.tile_pool(name="p3", bufs=2, space="PSUM") as p3pool,
                tc.tile_pool(name="o3", bufs=4) as o3pool,
            ):
                apools = (att, attsm, spsum, opsum, rspsum, oTpool)
                opools = (x2pool, wop, p3pool, o3pool)
                for bb in range(b):
                    _attn_batch(nc, p, apools, bb, qT_sb, kT_sb, v_sb, bv_sb,
                                maskU_sb, ones_sb, a2a_ins[bb])
                    if stages >= 3:
                        nc.gpsimd.collective_compute(
                            "AllToAll", mybir.AluOpType.bypass,
                            replica_groups=[list(range(n_cores))],
                            ins=[a2a_ins[bb][:, :].opt()],
                            outs=[a2a_outs[bb][:, :].opt()],
                        )
                        _outproj_batch(nc, p, opools, bb, a2a_outs[bb], wo,
                                       bo_sb, ones1, out)

    nc.compile()
    return nc


def _host_prep(x_norm, Wqkv, bqkv, Wout, bout, b, t, c, h, d, n_cores):
    """Build per-core input maps (numpy, bf16)."""
    HL = h // n_cores
    R = b * t
    perm = np.concatenate([np.arange(0, d, 2), np.arange(1, d, 2)])  # deinterleave

    XT = np.ascontiguousarray(x_norm.reshape(R, c).T.astype(NPBF16))
    inv_freq = 1.0 / (ROPE_BASE ** (np.arange(0, d, 2, dtype=np.float64) / d))
    ang = np.arange(t, dtype=np.float64)[None, :] * inv_freq[:, None]  # [d/2, t]
    cosT = np.concatenate([np.cos(ang), np.cos(ang)], axis=0).astype(NPBF16)
    sinT = np.concatenate([np.sin(ang), np.sin(ang)], axis=0).astype(NPBF16)
    # upper-triangular (incl diagonal) mask for the transposed P layout
    maskc = np.triu(np.ones((128, 128), dtype=np.float32)).astype(NPBF16)
    wo_b = np.ascontiguousarray(Wout.astype(NPBF16))
    bo_b = bout.reshape(1, c).astype(NPBF16)

    in_maps = []
    for i in range(n_cores):
        cols_q = np.concatenate([i * HL * d + hh * d + perm for hh in range(HL)])
        cols_k = cols_q + h * d
        cols_v = np.concatenate([2 * h * d + i * HL * d + hh * d + np.arange(d)
                                 for hh in range(HL)])
        wq_i = np.ascontiguousarray(Wqkv[:, cols_q].astype(NPBF16))
        wk_i = np.ascontiguousarray(Wqkv[:, cols_k].astype(NPBF16))
        wv_i = np.ascontiguousarray(Wqkv[:, cols_v].astype(NPBF16))
        bq_i = np.stack([bqkv[i * HL * d + hh * d + perm] for hh in range(HL)],
                        axis=1).astype(np.float32)
        bk_i = np.stack([bqkv[h * d + i * HL * d + hh * d + perm] for hh in range(HL)],
                        axis=1).astype(np.float32)
        bv_i = np.stack([bqkv[2 * h * d + i * HL * d + hh * d + np.arange(d)]
                         for hh in range(HL)], axis=1).astype(np.float32)
        in_maps.append({
            "xT": XT, "wq": wq_i, "wk": wk_i, "wv": wv_i,
            "bq": np.ascontiguousarray(bq_i), "bk": np.ascontiguousarray(bk_i),
            "bv": np.ascontiguousarray(bv_i),
            "wo": wo_b, "bo": bo_b, "cosT": cosT, "sinT": sinT, "maskc": maskc,
        })
    return in_maps


def _gather(parts, b, t, c, n_cores):
    """Core j's out rows are, for each batch bb, global rows
    [bb*t + j*seg, bb*t + (j+1)*seg) with seg = t // n_cores."""
    seg = t // n_cores
    R = b * t
    full = np.empty((R, c), dtype=np.float32)
    for j in range(n_cores):
        for bb in range(b):
            full[bb * t + j * seg: bb * t + (j + 1) * seg] = \
                parts[j][bb * seg:(bb + 1) * seg]
    return full.reshape(b, t, c)


_NC_CACHE = {}


def kernel(x_norm, Wqkv, bqkv, Wout, bout):
    b, t, c = x_norm.shape
    h = 16
    d = c // h
    key = (b, t, c)
    if key not in _NC_CACHE:
        _NC_CACHE[key] = build_nc(b, t, c, h, d, N_CORES)
    nc = _NC_CACHE[key]
    in_maps = _host_prep(np.asarray(x_norm, dtype=np.float32),
                         np.asarray(Wqkv, dtype=np.float32),
                         np.asarray(bqkv, dtype=np.float32),
                         np.asarray(Wout, dtype=np.float32),
                         np.asarray(bout, dtype=np.float32),
                         b, t, c, h, d, N_CORES)
    res = run_bass_kernel_spmd(nc, in_maps, core_ids=list(range(N_CORES)))
    parts = [np.asarray(res.results[i]["out"], dtype=np.float32) for i in range(N_CORES)]
    return _gather(parts, b, t, c, N_CORES)


# revision 20
# speedup vs baseline: 1.2549x; 1.0513x over previous
"""Trainium2 distributed kernel for nn_Attention (dense transformer attention block).

Strategy (8 NeuronCores, tensor-parallel over heads):
  - Host pre-transposes x_norm -> X^T [C, B*T] (bf16) and slices Wqkv columns
    per core (2 heads/core, deinterleaved RoPE feature order). RoPE sin/cos
    tables precomputed host-side.
  - Each core computes, in bf16 on the TensorEngine:
      1) Q^T/K^T (head-major, D on partitions) + V (natural) for its 2 heads,
         with bias + RoPE fused into the epilogue.
      2) Causal attention, "S^T" flash form without max-subtraction
         (scores ~ N(0,1)): for each K-tile jt and Tq-chunk c:
         S^T[tk, tq] = kT[jt].T @ qT-chunk -> exp (ACT, with 1/sqrt(D) scale,
         triangular mask on the diagonal block) -> P^T tile (SBUF bf16).
         Then two accumulating matmuls per tile: out^T += V[jt].T @ P^T and
         rowsums += ones.T @ P^T (broadcast row-sums on all 128 partitions).
         Normalize with a reciprocal multiply, add V-bias (P rows sum to 1).
      3) Per-batch AllToAll (1 MiB bf16) of out^T row-slices, overlapped with
         the next batch's attention.
      4) Per-batch local out-projection X2 @ Wout (+bout via rank-1 matmul).
  - Host reassembles the per-(core, batch) row pieces -> [B, T, C] fp32.
"""

import numpy as np
import ml_dtypes

import concourse.bass as bass
import concourse.mybir as mybir
import concourse.tile as tile
from concourse import bacc
from concourse.bass_utils import run_bass_kernel_spmd
from concourse.masks import make_identity

N_CORES = 8
B, T, C = 4, 2048, 2048
H, D = 16, 128
ROPE_BASE = 10000.0

BF16 = mybir.dt.bfloat16
F32 = mybir.dt.float32
NPBF16 = ml_dtypes.bfloat16


def _stage1(nc, tc, p, qT_sb, kT_sb, v_sb, wq_sb, wk_sb, wv_sb,
            bq_sb, bk_sb, cos_sb, sin_sb, xT):
    """QKV projection + bias + RoPE into resident SBUF."""
    RC, n_rc, KT, HL, t = p["RC"], p["n_rc"], p["KT"], p["HL"], p["t"]
    dma_engs = [nc.sync, nc.scalar]
    # ---- Q^T / K^T ----
    with (
        tc.tile_pool(name="xin_a", bufs=6) as xin,
        tc.tile_pool(name="ps_a", bufs=2 * 2 * HL, space="PSUM") as psa,
        tc.tile_pool(name="rope", bufs=4) as ropetmp,
    ):
        for rc in range(n_rc):
            r0 = rc * RC
            t0 = r0 % t
            psq = [psa.tile([128, RC], F32, tag="ps_qk", name=f"psq{rc}_{i}")
                   for i in range(2 * HL)]
            for kt in range(KT):
                xt = xin.tile([128, RC], BF16, tag="xt")
                dma_engs[kt % 2].dma_start(out=xt, in_=xT[kt * 128:(kt + 1) * 128, r0:r0 + RC])
                for hm in range(HL):
                    nc.tensor.matmul(psq[hm], lhsT=wq_sb[:, kt, hm * 128:(hm + 1) * 128],
                                     rhs=xt, start=(kt == 0), stop=(kt == KT - 1))
                    nc.tensor.matmul(psq[HL + hm], lhsT=wk_sb[:, kt, hm * 128:(hm + 1) * 128],
                                     rhs=xt, start=(kt == 0), stop=(kt == KT - 1))
            for which, (res, bias_sb) in enumerate(((qT_sb, bq_sb), (kT_sb, bk_sb))):
                for hm in range(HL):
                    dst = res[:, hm, r0:r0 + RC]
                    ps = psq[which * HL + hm]
                    nc.scalar.activation(out=dst, in_=ps,
                                         func=mybir.ActivationFunctionType.Identity,
                                         bias=bias_sb[:, hm:hm + 1], scale=1.0)
                    # RoPE in place: pairs (j, 64+j), angle t*w_j
                    x0 = res[0:64, hm, r0:r0 + RC]
                    x1 = res[64:128, hm, r0:r0 + RC]
                    rt = ropetmp.tile([128, RC], BF16, tag="rt")
                    nc.vector.tensor_mul(rt[0:64, :], x1, sin_sb[64:128, t0:t0 + RC])
                    nc.vector.tensor_mul(rt[64:128, :], x0, sin_sb[0:64, t0:t0 + RC])
                    nc.vector.tensor_mul(x0, x0, cos_sb[0:64, t0:t0 + RC])
                    nc.vector.tensor_sub(x0, x0, rt[0:64, :])
                    nc.vector.tensor_mul(x1, x1, cos_sb[64:128, t0:t0 + RC])
                    nc.vector.tensor_add(x1, x1, rt[64:128, :])
    # ---- V (natural layout) ----
    with (
        tc.tile_pool(name="xin_b", bufs=6) as xin,
        tc.tile_pool(name="ps_b", bufs=2 * (RC // 128), space="PSUM") as psb,
    ):
        for rc in range(n_rc):
            r0 = rc * RC
            psv = [psb.tile([128, p["HD"]], F32, tag="ps_v", name=f"psv{rc}_{i}")
                   for i in range(RC // 128)]
            for kt in range(KT):
                xt = xin.tile([128, RC], BF16, tag="xt")
                dma_engs[kt % 2].dma_start(out=xt, in_=xT[kt * 128:(kt + 1) * 128, r0:r0 + RC])
                for rs_ in range(RC // 128):
                    nc.tensor.matmul(psv[rs_], lhsT=xt[:, rs_ * 128:(rs_ + 1) * 128],
                                     rhs=wv_sb[:, kt, :], start=(kt == 0), stop=(kt == KT - 1))
            for rs_ in range(RC // 128):
                rt_ = (r0 // 128) + rs_
                nc.scalar.activation(out=v_sb[:, rt_, :], in_=psv[rs_],
                                     func=mybir.ActivationFunctionType.Copy, scale=1.0)


def _attn_batch(nc, p, pools, bb, qT_sb, kT_sb, v_sb, bv_sb, maskU_sb, ones_sb,
                a2a_in_b):
    """S^T-form causal attention for one batch (all local heads) -> a2a_in_b."""
    t, HL, HD, d = p["t"], p["HL"], p["HD"], p["d"]
    SCALE = p["SCALE"]
    NCH = t // 512                      # Tq chunks
    seg = t // N_CORES                  # rows per a2a slot
    att, attsm, rcpp, spsum, opsum, rspsum, oTpool = pools
    for hm in range(HL):
        qT_h = qT_sb[:, hm, bb * t:(bb + 1) * t]
        kT_h = kT_sb[:, hm, bb * t:(bb + 1) * t]
        oT = oTpool.tile([128, t], BF16, tag="oT", name=f"oT{bb}_{hm}")
        for c in range(NCH):
            tq0 = c * 512
            jt_max = 4 * (c + 1)
            psum_o = opsum.tile([128, 512], F32, tag="po", name=f"po{bb}_{hm}_{c}")
            psum_rs = rspsum.tile([128, 512], F32, tag="prs", name=f"prs{bb}_{hm}_{c}")
            for jt in range(jt_max):
                off = max(0, jt * 128 - tq0)
                w = 512 - off
                ps_st = spsum.tile([128, 512], F32, tag="s", name=f"st{bb}_{hm}_{c}_{jt}")
                nc.tensor.matmul(ps_st[:, off:512],
                                 lhsT=kT_h[:, jt * 128:(jt + 1) * 128],
                                 rhs=qT_h[:, tq0 + off:tq0 + 512],
                                 start=True, stop=True)
                pT = att.tile([128, 512], BF16, tag="pT", name=f"pT{bb}_{hm}_{c}_{jt}")
                if jt < 4 * c:
                    nc.scalar.activation(out=pT[:, :], in_=ps_st[:, :],
                                         func=mybir.ActivationFunctionType.Exp,
                                         scale=SCALE)
                else:
                    # diagonal block: cols [off, off+128) need the triu mask
                    tmp_d = attsm.tile([128, 128], BF16, tag="tmpd")
                    nc.scalar.activation(out=tmp_d, in_=ps_st[:, off:off + 128],
                                         func=mybir.ActivationFunctionType.Exp,
                                         scale=SCALE)
                    nc.vector.tensor_mul(pT[:, off:off + 128], tmp_d, maskU_sb)
                    if off + 128 < 512:
                        nc.scalar.activation(out=pT[:, off + 128:512],
                                             in_=ps_st[:, off + 128:512],
                                             func=mybir.ActivationFunctionType.Exp,
                                             scale=SCALE)
                nc.tensor.matmul(psum_o[:, off:512],
                                 lhsT=v_sb[:, (bb * t) // 128 + jt, hm * d:(hm + 1) * d],
                                 rhs=pT[:, off:512],
                                 start=(jt == 0), stop=(jt == jt_max - 1))
                nc.tensor.matmul(psum_rs[:, off:512], lhsT=ones_sb,
                                 rhs=pT[:, off:512],
                                 start=(jt == 0), stop=(jt == jt_max - 1))
            rcp = rcpp.tile([128, 512], F32, tag="rcp")
            nc.vector.reciprocal(out=rcp, in_=psum_rs)
            nc.vector.tensor_mul(oT[:, tq0:tq0 + 512], psum_o, rcp)
            nc.vector.tensor_scalar_add(oT[:, tq0:tq0 + 512], oT[:, tq0:tq0 + 512],
                                        bv_sb[:, hm:hm + 1])
        for sl in range(N_CORES):
            nc.gpsimd.dma_start(
                out=a2a_in_b[sl * HD + hm * d: sl * HD + (hm + 1) * d, :],
                in_=oT[:, sl * seg:(sl + 1) * seg])


def _outproj_batch(nc, p, pools, bb, a2a_out_b, wo, bo_sb, ones1, out):
    """Out-projection for this core's row-piece of batch bb."""
    c, KT = p["c"], p["KT"]
    seg = p["t"] // N_CORES             # rows in this piece
    x2pool, wop, p3pool, o3pool = pools
    x2t = x2pool.tile([128, KT, seg], BF16, tag="x2t", name=f"x2t{bb}")
    nc.sync.dma_start(out=x2t, in_=a2a_out_b[:, :].rearrange("(kt p) r -> p kt r", p=128))
    for nn_ in range(c // 512):
        wo_tiles = []
        for kt in range(KT):
            wt = wop.tile([128, 512], BF16, tag="wo", name=f"wo{bb}_{nn_}_{kt}")
            eng = nc.scalar if kt % 2 else nc.sync
            eng.dma_start(out=wt, in_=wo[kt * 128:(kt + 1) * 128,
                                         nn_ * 512:(nn_ + 1) * 512])
            wo_tiles.append(wt)
        MT = min(128, seg)
        for m in range(seg // MT):
            ps3 = p3pool.tile([MT, 512], F32, tag="ps3", name=f"ps3{bb}_{nn_}_{m}")
            for kt in range(KT):
                nc.tensor.matmul(ps3, lhsT=x2t[:, kt, m * MT:(m + 1) * MT],
                                 rhs=wo_tiles[kt], start=(kt == 0), stop=False)
            nc.tensor.matmul(ps3[0:MT, :], lhsT=ones1[0:1, 0:MT],
                             rhs=bo_sb[0:1, nn_ * 512:(nn_ + 1) * 512],
                             start=False, stop=True)
            o3 = o3pool.tile([MT, 512], F32, tag="o3", name=f"o3{bb}_{nn_}_{m}")
            nc.scalar.activation(out=o3, in_=ps3,
                                 func=mybir.ActivationFunctionType.Copy, scale=1.0)
            nc.sync.dma_start(out=out[bb * seg + m * MT:bb * seg + (m + 1) * MT,
                                      nn_ * 512:(nn_ + 1) * 512], in_=o3)


def build_nc(b=B, t=T, c=C, h=H, d=D, n_cores=N_CORES, stages=3):
    HL = h // n_cores          # heads per core
    R = b * t                  # total rows
    RS = R // n_cores          # rows per core overall
    RC = 512                   # row-chunk for stage 1
    p = dict(b=b, t=t, c=c, h=h, d=d, HL=HL, R=R, RS=RS, RC=RC,
             n_rc=R // RC, KT=c // 128, NQT=t // 128, HD=HL * d,
             SCALE=1.0 / float(np.sqrt(d)))
    seg = t // n_cores

    nc = bacc.Bacc(None, target_bir_lowering=False, debug=False,
                   num_devices=n_cores)

    xT = nc.declare_dram_parameter("xT", [c, R], BF16, isOutput=False)
    wq = nc.declare_dram_parameter("wq", [c, p["HD"]], BF16, isOutput=False)
    wk = nc.declare_dram_parameter("wk", [c, p["HD"]], BF16, isOutput=False)
    wv = nc.declare_dram_parameter("wv", [c, p["HD"]], BF16, isOutput=False)
    bq = nc.declare_dram_parameter("bq", [128, HL], F32, isOutput=False)
    bk = nc.declare_dram_parameter("bk", [128, HL], F32, isOutput=False)
    bv = nc.declare_dram_parameter("bv", [128, HL], F32, isOutput=False)
    wo = nc.declare_dram_parameter("wo", [c, c], BF16, isOutput=False)
    bo = nc.declare_dram_parameter("bo", [1, c], BF16, isOutput=False)
    cosT = nc.declare_dram_parameter("cosT", [128, t], BF16, isOutput=False)
    sinT = nc.declare_dram_parameter("sinT", [128, t], BF16, isOutput=False)
    maskc = nc.declare_dram_parameter("maskc", [128, 128], BF16, isOutput=False)
    out = nc.declare_dram_parameter("out", [RS, c], F32, isOutput=True)

    with tile.TileContext(nc) as tc:
        with (
            tc.tile_pool(name="consts", bufs=1) as consts,
            tc.tile_pool(name="qkvres", bufs=1) as qkvres,
            tc.tile_pool(name="dram", bufs=1, space="DRAM") as dram,
        ):
            # ---- constants into SBUF ----
            wq_sb = consts.tile([128, p["KT"], p["HD"]], BF16, tag="wq")
            wk_sb = consts.tile([128, p["KT"], p["HD"]], BF16, tag="wk")
            wv_sb = consts.tile([128, p["KT"], p["HD"]], BF16, tag="wv")
            nc.sync.dma_start(out=wq_sb, in_=wq[:, :].rearrange("(kt p) n -> p kt n", p=128))
            nc.sync.dma_start(out=wk_sb, in_=wk[:, :].rearrange("(kt p) n -> p kt n", p=128))
            nc.sync.dma_start(out=wv_sb, in_=wv[:, :].rearrange("(kt p) n -> p kt n", p=128))
            bq_sb = consts.tile([128, HL], F32, tag="bq")
            bk_sb = consts.tile([128, HL], F32, tag="bk")
            bv_sb = consts.tile([128, HL], F32, tag="bv")
            nc.sync.dma_start(out=bq_sb, in_=bq[:, :])
            nc.sync.dma_start(out=bk_sb, in_=bk[:, :])
            nc.sync.dma_start(out=bv_sb, in_=bv[:, :])
            cos_sb = consts.tile([128, t], BF16, tag="cos")
            sin_sb = consts.tile([128, t], BF16, tag="sin")
            nc.sync.dma_start(out=cos_sb, in_=cosT[:, :])
            nc.sync.dma_start(out=sin_sb, in_=sinT[:, :])
            maskU_sb = consts.tile([128, 128], BF16, tag="mask")
            nc.sync.dma_start(out=maskU_sb, in_=maskc[:, :])
            bo_sb = consts.tile([1, c], BF16, tag="bo")
            nc.sync.dma_start(out=bo_sb, in_=bo[:, :])
            ones1 = consts.tile([1, 128], BF16, tag="ones1")
            nc.vector.memset(ones1, 1.0)
            ones_sb = consts.tile([128, 128], BF16, tag="ones128")
            nc.vector.memset(ones_sb, 1.0)

            # ---- resident QKV (bf16) ----
            qT_sb = qkvres.tile([128, HL, p["R"]], BF16, tag="qT")   # [d, h, row]
            kT_sb = qkvres.tile([128, HL, p["R"]], BF16, tag="kT")
            v_sb = qkvres.tile([128, p["R"] // 128, p["HD"]], BF16, tag="v")

            _stage1(nc, tc, p, qT_sb, kT_sb, v_sb, wq_sb, wk_sb, wv_sb,
                    bq_sb, bk_sb, cos_sb, sin_sb, xT)

            a2a_ins = []
            a2a_outs = []
            for bb in range(b):
                a2a_ins.append(dram.tile([n_cores * p["HD"], seg], BF16,
                                         tag=f"a2a_in{bb}", name=f"a2a_in{bb}"))
                a2a_outs.append(dram.tile([n_cores * p["HD"], seg], BF16,
                                          tag=f"a2a_out{bb}", name=f"a2a_out{bb}"))

            with (
                tc.tile_pool(name="att", bufs=3) as att,
                tc.tile_pool(name="attsm", bufs=4) as attsm,
                tc.tile_pool(name="rcpp", bufs=2) as rcpp,
                tc.tile_pool(name="spsum", bufs=2, space="PSUM") as spsum,
                tc.tile_pool(name="opsum", bufs=2, space="PSUM") as opsum,
                tc.tile_pool(name="rspsum", bufs=2, space="PSUM") as rspsum,
                tc.tile_pool(name="oTp", bufs=2) as oTpool,
                tc.tile_pool(name="x2", bufs=1) as x2pool,
                tc.tile_pool(name="wop", bufs=24) as wop,
                tc.tile_pool(name="p3", bufs=2, space="PSUM") as p3pool,
                tc.tile_pool(name="o3", bufs=3) as o3pool,
            ):
                apools = (att, attsm, rcpp, spsum, opsum, rspsum, oTpool)
                opools = (x2pool, wop, p3pool, o3pool)
                for bb in range(b):
                    _attn_batch(nc, p, apools, bb, qT_sb, kT_sb, v_sb, bv_sb,
                                maskU_sb, ones_sb, a2a_ins[bb])
                    if stages >= 3:
                        nc.gpsimd.collective_compute(
                            "AllToAll", mybir.AluOpType.bypass,
                            replica_groups=[list(range(n_cores))],
                            ins=[a2a_ins[bb][:, :].opt()],
                            outs=[a2a_outs[bb][:, :].opt()],
                        )
                        if bb > 0:
                            _outproj_batch(nc, p, opools, bb - 1, a2a_outs[bb - 1],
                                           wo, bo_sb, ones1, out)
                if stages >= 3:
                    _outproj_batch(nc, p, opools, b - 1, a2a_outs[b - 1],
                                   wo, bo_sb, ones1, out)

    nc.compile()
    return nc


def _host_prep(x_norm, Wqkv, bqkv, Wout, bout, b, t, c, h, d, n_cores):
    """Build per-core input maps (numpy, bf16)."""
    HL = h // n_cores
    R = b * t
    perm = np.concatenate([np.arange(0, d, 2), np.arange(1, d, 2)])  # deinterleave

    XT = np.ascontiguousarray(x_norm.reshape(R, c).T.astype(NPBF16))
    inv_freq = 1.0 / (ROPE_BASE ** (np.arange(0, d, 2, dtype=np.float64) / d))
    ang = np.arange(t, dtype=np.float64)[None, :] * inv_freq[:, None]  # [d/2, t]
    cosT = np.concatenate([np.cos(ang), np.cos(ang)], axis=0).astype(NPBF16)
    sinT = np.concatenate([np.sin(ang), np.sin(ang)], axis=0).astype(NPBF16)
    # upper-triangular (incl diagonal) mask for the transposed P layout
    maskc = np.triu(np.ones((128, 128), dtype=np.float32)).astype(NPBF16)
    wo_b = np.ascontiguousarray(Wout.astype(NPBF16))
    bo_b = bout.reshape(1, c).astype(NPBF16)

    in_maps = []
    for i in range(n_cores):
        cols_q = np.concatenate([i * HL * d + hh * d + perm for hh in range(HL)])
        cols_k = cols_q + h * d
        cols_v = np.concatenate([2 * h * d + i * HL * d + hh * d + np.arange(d)
                                 for hh in range(HL)])
        wq_i = np.ascontiguousarray(Wqkv[:, cols_q].astype(NPBF16))
        wk_i = np.ascontiguousarray(Wqkv[:, cols_k].astype(NPBF16))
        wv_i = np.ascontiguousarray(Wqkv[:, cols_v].astype(NPBF16))
        bq_i = np.stack([bqkv[i * HL * d + hh * d + perm] for hh in range(HL)],
                        axis=1).astype(np.float32)
        bk_i = np.stack([bqkv[h * d + i * HL * d + hh * d + perm] for hh in range(HL)],
                        axis=1).astype(np.float32)
        bv_i = np.stack([bqkv[2 * h * d + i * HL * d + hh * d + np.arange(d)]
                         for hh in range(HL)], axis=1).astype(np.float32)
        in_maps.append({
            "xT": XT, "wq": wq_i, "wk": wk_i, "wv": wv_i,
            "bq": np.ascontiguousarray(bq_i), "bk": np.ascontiguousarray(bk_i),
            "bv": np.ascontiguousarray(bv_i),
            "wo": wo_b, "bo": bo_b, "cosT": cosT, "sinT": sinT, "maskc": maskc,
        })
    return in_maps


def _gather(parts, b, t, c, n_cores):
    """Core j's out rows are, for each batch bb, global rows
    [bb*t + j*seg, bb*t + (j+1)*seg) with seg = t // n_cores."""
    seg = t // n_cores
    R = b * t
    full = np.empty((R, c), dtype=np.float32)
    for j in range(n_cores):
        for bb in range(b):
            full[bb * t + j * seg: bb * t + (j + 1) * seg] = \
                parts[j][bb * seg:(bb + 1) * seg]
    return full.reshape(b, t, c)


_NC_CACHE = {}


def kernel(x_norm, Wqkv, bqkv, Wout, bout):
    b, t, c = x_norm.shape
    h = 16
    d = c // h
    key = (b, t, c)
    if key not in _NC_CACHE:
        _NC_CACHE[key] = build_nc(b, t, c, h, d, N_CORES)
    nc = _NC_CACHE[key]
    in_maps = _host_prep(np.asarray(x_norm, dtype=np.float32),
                         np.asarray(Wqkv, dtype=np.float32),
                         np.asarray(bqkv, dtype=np.float32),
                         np.asarray(Wout, dtype=np.float32),
                         np.asarray(bout, dtype=np.float32),
                         b, t, c, h, d, N_CORES)
    res = run_bass_kernel_spmd(nc, in_maps, core_ids=list(range(N_CORES)))
    parts = [np.asarray(res.results[i]["out"], dtype=np.float32) for i in range(N_CORES)]
    return _gather(parts, b, t, c, N_CORES)
